# revision 16
# baseline (speedup 1.0000x reference)
"""Trainium2 Bass kernel for nn_AEFIT (ragged NaN-compaction VAE loss).

Strategy (pure data-parallel over the batch, 8 NeuronCores):
  - per-row compaction of finite values done on-device:
      rank = inclusive-cumsum(att) trick  ->  int16 scatter indices
      gpsimd.local_scatter writes compacted row (zero tail for free)
  - encoder runs in "transposed space" (features on partitions, rows on
    the moving free dim) so no transpose of the big activations is needed;
    the decoder returns to row-major via lhsT = d1T.
  - every loss term is reduced on-device to per-partition partial sums;
    the host combines 8 cores' partials into the scalar output.

Host-visible contract:  kernel(**inputs) -> np.float32 scalar, matching
reference.reference(**setup_inputs()).
"""

import sys
import os
import math
from contextlib import ExitStack

import numpy as np

for _p in ("/opt/trn_rl_repo",):
    if _p not in sys.path:
        sys.path.insert(0, _p)

import concourse.bass as bass
import concourse.bacc as bacc
import concourse.tile as tile
from concourse import mybir
from concourse.bass_utils import run_bass_kernel_spmd

# Pin every activation to the one LUT set that contains exp+ln+relu+square;
# emptying the other sets (indices preserved) stops LoadActFuncSet ping-pong.
_orig_gat = bacc.get_activation_tables


def _gat_pinned(arch):
    tabs = _orig_gat(arch)
    if "natural_log_exp_and_others" in tabs:
        tabs = {k: (v if k == "natural_log_exp_and_others" else set())
                for k, v in tabs.items()}
    return tabs


bacc.get_activation_tables = _gat_pinned

AluOp = mybir.AluOpType
Act = mybir.ActivationFunctionType
dt = mybir.dt

NCORES = 8
D = 1024
L = 512
V = 128
DG = 10          # generative hidden width
LOG2PI = float(np.log(2.0 * np.pi))
LN2 = float(np.log(2.0))

_np_bf16 = dt.np(dt.bfloat16)

_GRAPH_CACHE = {}
_LAST_IN_MAPS = None


# --------------------------------------------------------------------------
# graph builder
# --------------------------------------------------------------------------
def _build(B_core: int, k_chunks: int):
    """Build the per-core Bass graph. B_core rows, contraction depth
    K_EFF = 128*k_chunks for the compacted encoder matmul."""
    NT = B_core // 128           # number of 128-row subtiles
    NBLK = NT // 4               # 512-row blocks
    K_EFF = 128 * k_chunks
    assert NT % 4 == 0

    nc = bacc.Bacc("TRN2", target_bir_lowering=False, debug=False,
                   num_devices=NCORES)

    def param(name, shape, dtype):
        return nc.dram_tensor(name, list(shape), dtype, kind="ExternalInput").ap()

    def out_param(name, shape, dtype):
        return nc.dram_tensor(name, list(shape), dtype, kind="ExternalOutput").ap()

    xy_e = param("xy", (B_core, D), dt.bfloat16)
    att_e = param("att", (B_core, L), dt.uint8)
    eps_e = param("eps", (B_core, V), dt.float32)
    A_e = param("A", (K_EFF, D), dt.bfloat16)          # diag(w1) @ W2, trimmed
    W3_e = param("W3", (D, 2 * V), dt.bfloat16)
    Wg1_e = param("Wg1", (V, DG), dt.bfloat16)
    Wg2_e = param("Wg2", (DG, D), dt.bfloat16)
    b2s_e = param("b2s", (128, 8), dt.float32)         # b2 reshaped (8,128).T
    b3s_e = param("b3s", (128, 2), dt.float32)         # [b3_lo, 0.5*b3_hi]
    bg1_e = param("bg1s", (DG, 1), dt.float32)
    idbf_e = param("ident_bf", (128, 128), dt.bfloat16)
    idf32_e = param("ident_f32", (128, 128), dt.float32)

    acc_e = out_param("acc", (128, NT, 8), dt.float32)
    p_e = out_param("pmat", (DG, D), dt.float32)

    with tile.TileContext(nc) as tc, ExitStack() as ctx:
        const = ctx.enter_context(tc.tile_pool(name="const", bufs=1))
        io = ctx.enter_context(tc.tile_pool(name="io", bufs=2))
        rk = ctx.enter_context(tc.tile_pool(name="rk", bufs=2))
        hold = ctx.enter_context(tc.tile_pool(name="hold", bufs=2))
        blk = ctx.enter_context(tc.tile_pool(name="blk", bufs=2))
        scratch = ctx.enter_context(tc.tile_pool(name="scratch", bufs=2))
        dead = ctx.enter_context(tc.tile_pool(name="dead", bufs=1))
        # PSUM budget is 8 banks: P(2) + trans(2) + mm(2) + XY(2)
        pp = ctx.enter_context(tc.tile_pool(name="pp", bufs=2, space="PSUM"))
        pp_mm = ctx.enter_context(tc.tile_pool(name="pp_mm", bufs=2, space="PSUM"))
        pp_xy = ctx.enter_context(tc.tile_pool(name="pp_xy", bufs=2, space="PSUM"))
        pp_p = ctx.enter_context(tc.tile_pool(name="pp_p", bufs=1, space="PSUM"))

        # ---- constants into SBUF ----
        A_sb = const.tile([128, k_chunks, D], dt.bfloat16)
        for k in range(k_chunks):
            nc.sync.dma_start(out=A_sb[:, k, :], in_=A_e[128 * k:128 * (k + 1), :])
        W3_sb = const.tile([128, 8, 2 * V], dt.bfloat16)
        for k in range(8):
            nc.sync.dma_start(out=W3_sb[:, k, :], in_=W3_e[128 * k:128 * (k + 1), :])
        Wg1_sb = const.tile([V, DG], dt.bfloat16)
        nc.sync.dma_start(out=Wg1_sb[:], in_=Wg1_e[:])
        Wg2_sb = const.tile([DG, D], dt.bfloat16)
        nc.sync.dma_start(out=Wg2_sb[:], in_=Wg2_e[:])
        b2s = const.tile([128, 8], dt.float32)
        nc.sync.dma_start(out=b2s[:], in_=b2s_e[:])
        b3s = const.tile([128, 2], dt.float32)
        nc.sync.dma_start(out=b3s[:], in_=b3s_e[:])
        bg1s = const.tile([DG, 1], dt.float32)
        nc.sync.dma_start(out=bg1s[:], in_=bg1_e[:])
        id_bf = const.tile([128, 128], dt.bfloat16)
        nc.sync.dma_start(out=id_bf[:], in_=idbf_e[:])
        id_f32 = const.tile([128, 128], dt.float32)
        nc.sync.dma_start(out=id_f32[:], in_=idf32_e[:])

        acc = const.tile([128, NT, 8], dt.float32)
        nc.vector.memset(acc[:], 0.0)

        # persistent PSUM accumulators for P = sum d1^T @ xc  ([10, 1024])
        P_ps = []
        for h in range(2):
            P_half = pp_p.tile([DG, 512], dt.float32, tag=f"P{h}", name=f"P{h}")
            P_ps.append(P_half)

        for b in range(NBLK):
            r0 = 512 * b
            xyB = io.tile([128, 4, D], dt.bfloat16, tag="xy")
            nc.sync.dma_start(
                out=xyB[:],
                in_=xy_e[r0:r0 + 512, :].rearrange("(s p) d -> p s d", p=128))
            attB = io.tile([128, 4, L], dt.uint8, tag="att")
            nc.sync.dma_start(
                out=attB[:],
                in_=att_e[r0:r0 + 512, :].rearrange("(s p) d -> p s d", p=128))
            epsB = io.tile([128, 4, V], dt.float32, tag="eps")
            nc.sync.dma_start(
                out=epsB[:],
                in_=eps_e[r0:r0 + 512, :].rearrange("(s p) d -> p s d", p=128))

            cT_sb = blk.tile([128, k_chunks, 512], dt.bfloat16, tag="cT")
            epsT_blk = blk.tile([128, 512], dt.float32, tag="epsT")

            # ---- ranks (scan per subtile, everything else block-wide) ----
            s1_blk = rk.tile([128, 4, L], dt.float16, tag="s1")
            for s in range(4):
                st = 4 * b + s
                nc.vector.tensor_tensor_scan(s1_blk[:, s, :], attB[:, s, :],
                                             attB[:, s, :], 0.0,
                                             AluOp.add, AluOp.bypass)
                nc.vector.tensor_copy(acc[:, st, 0:1], s1_blk[:, s, L - 1:L])
            m1_blk = rk.tile([128, 4, L], dt.float16, tag="mm16")
            nc.vector.tensor_tensor(m1_blk[:], s1_blk[:], attB[:], AluOp.mult)
            idx_blk = rk.tile([128, 4, D], dt.int16, tag="idx")
            nc.vector.tensor_scalar(idx_blk[:, :, 0:L], m1_blk[:], -1.0, None,
                                    AluOp.add)
            m2_blk = rk.tile([128, 4, L], dt.float16, tag="mm16")
            for s in range(4):
                st = 4 * b + s
                nc.vector.scalar_tensor_tensor(m2_blk[:, s, :], s1_blk[:, s, :],
                                               acc[:, st, 0:1], attB[:, s, :],
                                               AluOp.add, AluOp.mult)
            nc.vector.tensor_scalar(idx_blk[:, :, L:D], m2_blk[:], -1.0, None,
                                    AluOp.add)

            # ---- compaction + xc (xy already bf16 from host) ----
            c_blk = rk.tile([128, 4, K_EFF], dt.bfloat16, tag="c")
            for s in range(4):
                nc.gpsimd.local_scatter(c_blk[:, s, :], xyB[:, s, :],
                                        idx_blk[:, s, :], 128, K_EFF, D)
            xc_blk = hold.tile([128, 4, D], dt.bfloat16, tag="xc")
            nc.gpsimd.memset(xc_blk[:], 0.0)
            a_ap = attB[:]
            m40_4 = bass.AP(tensor=a_ap.tensor, offset=a_ap.offset,
                            ap=[a_ap.ap[0], a_ap.ap[1], [0, 2], a_ap.ap[2]])
            nc.vector.copy_predicated(
                xc_blk[:].rearrange("p s (a l) -> p s a l", a=2), m40_4,
                xyB[:].rearrange("p s (a l) -> p s a l", a=2))

            # ---- sum c^2 over the whole block (== sum xc^2) ----
            c2_sc = dead.tile([128, 4, K_EFF], dt.bfloat16, tag="dead", name="c2_sc")
            nc.scalar.activation(c2_sc[:], c_blk[:], Act.Square,
                                 accum_out=acc[:, 4 * b, 2:3])

            # ---- transposes ----
            for s in range(4):
                cT_ps = pp.tile([128, k_chunks * 128], dt.bfloat16, tag="tp",
                                name="cTps")
                for k in range(k_chunks):
                    nc.tensor.transpose(cT_ps[:, 128 * k:128 * (k + 1)],
                                        c_blk[:, s, 128 * k:128 * (k + 1)],
                                        id_bf[:])
                nc.scalar.activation(
                    cT_sb[:, :, 128 * s:128 * (s + 1)],
                    cT_ps[:].rearrange("p (k c) -> p k c", k=k_chunks),
                    Act.Copy)
                epsT_ps = pp.tile([128, V], dt.float32, tag="tp", name="epsTps")
                nc.tensor.transpose(epsT_ps[:], epsB[:, s, :], id_f32[:])
                nc.scalar.activation(epsT_blk[:, 128 * s:128 * (s + 1)],
                                     epsT_ps[:], Act.Copy)

            # ================= block compute (512 rows) =================
            hT_sb = blk.tile([128, 8, 512], dt.bfloat16, tag="hT")
            for f in range(8):
                vT_ps = pp_mm.tile([128, 512], dt.float32, tag="mm")
                for k in range(k_chunks):
                    nc.tensor.matmul(vT_ps[:],
                                     A_sb[:, k, 128 * f:128 * (f + 1)],
                                     cT_sb[:, k, :],
                                     start=(k == 0), stop=(k == k_chunks - 1))
                nc.scalar.activation(hT_sb[:, f, :], vT_ps[:], Act.Relu,
                                     bias=b2s[:, f:f + 1])
            encT_ps = []
            for f2 in range(2):
                e_ps = pp_mm.tile([128, 512], dt.float32, tag="mm",
                                  name=f"encT{f2}")
                for k in range(8):
                    nc.tensor.matmul(e_ps[:],
                                     W3_sb[:, k, 128 * f2:128 * (f2 + 1)],
                                     hT_sb[:, k, :],
                                     start=(k == 0), stop=(k == 7))
                encT_ps.append(e_ps)
            meanT = blk.tile([128, 512], dt.float32, tag="meanT")
            nc.scalar.activation(meanT[:], encT_ps[0][:], Act.Identity,
                                 bias=b3s[:, 0:1])
            sigT = blk.tile([128, 512], dt.float32, tag="sigT")
            nc.scalar.activation(sigT[:], encT_ps[1][:], Act.Exp,
                                 bias=b3s[:, 1:2], scale=0.5)
            nc.vector.tensor_reduce(acc[:, 4 * b, 5:6], encT_ps[1][:],
                                    mybir.AxisListType.X, AluOp.add)
            sT_a = blk.tile([128, 512], dt.float32, tag="sTa")
            nc.vector.tensor_tensor(sT_a[:], epsT_blk[:], sigT[:], AluOp.mult)
            sT16 = blk.tile([128, 512], dt.bfloat16, tag="sT16")
            nc.vector.tensor_tensor(sT16[:], sT_a[:], meanT[:], AluOp.add)
            sq_s = dead.tile([128, 512], dt.float32, tag="dead", name="sq_s")
            nc.scalar.activation(sq_s[:], sT16[:], Act.Square,
                                 accum_out=acc[:, 4 * b, 4:5])

            d1T_ps = pp_mm.tile([DG, 512], dt.float32, tag="mm", name="d1T")
            nc.tensor.matmul(d1T_ps[:], Wg1_sb[:], sT16[:], start=True,
                             stop=True)
            d1T_sb = blk.tile([DG, 512], dt.bfloat16, tag="d1Tsb")
            nc.scalar.activation(d1T_sb[:], d1T_ps[:], Act.Relu, bias=bg1s[:])

            # all four d1 heads into one PSUM bank, single relu
            d1_ps = pp_mm.tile([128, 4 * DG], dt.float32, tag="mm", name="d1")
            for s in range(4):
                nc.tensor.matmul(d1_ps[:, DG * s:DG * (s + 1)],
                                 sT16[:, 128 * s:128 * (s + 1)],
                                 Wg1_sb[:], start=True, stop=True,
                                 skip_group_check=True)
            d1_sb = scratch.tile([128, 4, DG], dt.bfloat16, tag="d1sb")
            nc.scalar.activation(
                d1_sb[:], d1_ps[:].rearrange("p (s g) -> p s g", s=4),
                Act.Relu)

            t16_blk = scratch.tile([128, 4, D], dt.bfloat16, tag="t16")
            for s in range(4):
                st = 4 * b + s
                for h in range(2):
                    # XY half-tiles (1 bank, double-buffered): PE can start
                    # the next half while DVE masks the previous one
                    XY_ps = pp_xy.tile([128, 512], dt.float32, tag="XY",
                                       name="XYh")
                    nc.tensor.matmul(XY_ps[:],
                                     d1T_sb[:, 128 * s:128 * (s + 1)],
                                     Wg2_sb[:, 512 * h:512 * (h + 1)],
                                     start=True, stop=True)
                    nc.tensor.matmul(P_ps[h][:], d1_sb[:, s, :],
                                     xc_blk[:, s, 512 * h:512 * (h + 1)],
                                     start=(st == 0), stop=(st == NT - 1),
                                     skip_group_check=True)
                    # both D-halves share the same [B, L] mask (m40 tiling)
                    nc.vector.tensor_tensor(
                        t16_blk[:, s, 512 * h:512 * (h + 1)],
                        XY_ps[:], attB[:, s, :], AluOp.mult)

            # block-wide softplus sum: ln(exp(t)+1), and sum t^2
            eu_sc = dead.tile([128, 4, D], dt.bfloat16, tag="eusc", name="eu_sc")
            nc.scalar.activation(eu_sc[:], t16_blk[:], Act.Exp)
            sp_sc = dead.tile([128, 4, D], dt.bfloat16, tag="dead", name="sp_sc")
            nc.scalar.activation(sp_sc[:], eu_sc[:], Act.Ln, bias=1.0,
                                 accum_out=acc[:, 4 * b, 1:2])
            t2_sc = dead.tile([128, 4, D], dt.bfloat16, tag="dead", name="t2_sc")
            nc.vector.scalar_tensor_tensor(t2_sc[:], t16_blk[:], 1.0,
                                           t16_blk[:], AluOp.mult, AluOp.mult,
                                           accum_out=acc[:, 4 * b, 6:7])

        # ---- outputs ----
        P_sb = const.tile([DG, D], dt.float32)
        for h in range(2):
            nc.scalar.activation(P_sb[:, 512 * h:512 * (h + 1)], P_ps[h][:],
                                 Act.Copy)
        nc.sync.dma_start(out=p_e[:], in_=P_sb[:])
        nc.sync.dma_start(out=acc_e[:], in_=acc[:])

    nc.compile()
    return nc


def _get_graph(B_core, k_chunks):
    key = (B_core, k_chunks)
    if key not in _GRAPH_CACHE:
        _GRAPH_CACHE[key] = _build(B_core, k_chunks)
    return _GRAPH_CACHE[key]


# --------------------------------------------------------------------------
# exact numpy fallback (only used for weight configs the device path
# doesn't specialize for; never triggered by the reference setup)
# --------------------------------------------------------------------------
def _numpy_exact(xy, att, eps, w1, b1, W2, b2, W3, b3, Wg1, bg1, Wg2, bg2):
    B, Dd = xy.shape
    Ld = Dd // 2
    m = np.isfinite(xy)
    xc = np.where(m, xy, 0.0).astype(np.float32)
    order = np.argsort(~m, axis=1, kind="stable")
    c = np.take_along_axis(xc, order, axis=1)
    r = m.sum(1, keepdims=True)
    y = np.where(np.arange(Dd)[None, :] < r, c * w1 + b1, 0.0).astype(np.float32)
    h = np.maximum(y @ W2 + b2, 0.0)
    enc = h @ W3 + b3
    mean, logv = enc[:, :enc.shape[1] // 2], enc[:, enc.shape[1] // 2:]
    s = eps * np.exp(0.5 * logv) + mean
    d1 = np.maximum(s @ Wg1 + bg1, 0.0)
    XY = d1 @ Wg2 + bg2
    attf = att.astype(np.float32)
    x1, x2 = xc[:, :Ld], xc[:, Ld:]
    X1, X2 = XY[:, :Ld], XY[:, Ld:]
    per_pt = 0.5 * ((x1 - X1) ** 2 + (x2 - X2) ** 2)
    l0 = (per_pt * attf).sum() / attf.sum()
    m40 = np.tile(attf, (1, 2))
    ce = np.maximum(XY, 0) - XY * xc + np.log1p(np.exp(-np.abs(XY)))
    logpx = -(ce * m40).sum(1)
    logpz = (-0.5 * (s ** 2 + LOG2PI)).sum(1)
    logqz = (-0.5 * ((s - mean) ** 2 * np.exp(-logv) + logv + LOG2PI)).sum(1)
    l_vae = -np.mean(logpx + logpz - logqz)
    return np.float32(l_vae + np.exp(l0))


# --------------------------------------------------------------------------
# host entry point
# --------------------------------------------------------------------------
def kernel(xy, att, eps, w1, b1, W2, b2, W3, b3, Wg1, bg1, Wg2, bg2):
    xy = np.asarray(xy, np.float32)
    att = np.asarray(att)
    eps = np.asarray(eps, np.float32)
    w1 = np.asarray(w1, np.float32)
    b1 = np.asarray(b1, np.float32)
    W2 = np.asarray(W2, np.float32)
    b2 = np.asarray(b2, np.float32)
    W3 = np.asarray(W3, np.float32)
    b3 = np.asarray(b3, np.float32)
    Wg1 = np.asarray(Wg1, np.float32)
    bg1 = np.asarray(bg1, np.float32)
    Wg2 = np.asarray(Wg2, np.float32)
    bg2 = np.asarray(bg2, np.float32)

    B = xy.shape[0]
    if np.any(b1) or np.any(bg1 != 0) or np.any(bg2):
        # device fast path folds these as zeros; exact fallback otherwise
        return _numpy_exact(xy, att, eps, w1, b1, W2, b2, W3, b3,
                            Wg1, bg1, Wg2, bg2)

    attu8 = att.astype(np.uint8)
    n_row = attu8.sum(1, dtype=np.int64)
    rmax = int(2 * n_row.max()) if B else 0
    k_chunks = max(1, min(8, -(-max(rmax, 1) // 128)))
    K_EFF = 128 * k_chunks

    B_core = B // NCORES
    nc = _get_graph(B_core, k_chunks)

    A = (w1[:K_EFF, None] * W2[:K_EFF]).astype(_np_bf16)
    b2s = np.ascontiguousarray(b2.reshape(8, 128).T.astype(np.float32))
    b3s = np.stack([b3[:V], 0.5 * b3[V:]], axis=1).astype(np.float32)
    b3s = np.ascontiguousarray(b3s)
    shared = {
        "A": np.ascontiguousarray(A),
        "W3": W3.astype(_np_bf16),
        "Wg1": Wg1.astype(_np_bf16),
        "Wg2": Wg2.astype(_np_bf16),
        "b2s": b2s,
        "b3s": b3s,
        "bg1s": np.ascontiguousarray(bg1.reshape(DG, 1).astype(np.float32)),
        "ident_bf": np.eye(128, dtype=np.float32).astype(_np_bf16),
        "ident_f32": np.eye(128, dtype=np.float32),
    }
    in_maps = []
    for i in range(NCORES):
        sl = slice(i * B_core, (i + 1) * B_core)
        m = dict(shared)
        m["xy"] = np.ascontiguousarray(xy[sl]).astype(_np_bf16)
        m["att"] = np.ascontiguousarray(attu8[sl])
        m["eps"] = np.ascontiguousarray(eps[sl])
        in_maps.append(m)

    global _LAST_IN_MAPS
    _LAST_IN_MAPS = in_maps
    res = run_bass_kernel_spmd(nc, in_maps, list(range(NCORES)))
    accs = np.stack([np.asarray(r["acc"], np.float64) for r in res.results])
    pmat = np.sum([np.asarray(r["pmat"], np.float64) for r in res.results],
                  axis=0)

    S_att = accs[..., 0].sum()
    S_sp = accs[..., 1].sum()
    S_c2 = accs[..., 2].sum()
    S_eps2 = float((eps.astype(np.float64) ** 2).sum())
    S_s2 = accs[..., 4].sum()
    S_enc2 = accs[..., 5].sum()
    S_t2 = accs[..., 6].sum()
    S_logv = S_enc2 + B * float(b3[V:].sum())

    B_term = float((Wg2.astype(np.float64) * pmat).sum())
    sum_sp_masked = S_sp - (B * D - 2.0 * S_att) * LN2
    sum_ce = sum_sp_masked - B_term
    S_d2 = S_c2 + S_t2 - 2.0 * B_term
    l0 = 0.5 * S_d2 / S_att
    sum_logpx = -sum_ce
    sum_logpz = -0.5 * (S_s2 + B * V * LOG2PI)
    sum_logqz = -0.5 * (S_eps2 + S_logv + B * V * LOG2PI)
    l_vae = -(sum_logpx + sum_logpz - sum_logqz) / B
    return np.float32(l_vae + math.exp(l0))
